# revision 1
# baseline (speedup 1.0000x reference)
"""Trainium2 Bass kernel for the 2-layer GraphSAGE encoder (mean aggregation).

Computation (see reference):
  h   = relu(mean_agg(relu(x)[src] by dst) @ W_l1 + b_l1 + x @ W_r1)
  out =      mean_agg(h[src]       by dst) @ W_l2 + b_l2 + h @ W_r2

Distribution: edges are partitioned across the 8 cores by destination
shard (12500 nodes each).  Within a core, edges are grouped by
(dst window of 128 nodes, src bank of 4) and padded to chunks of 128.
Messages are fetched with batched dma_gather (bf16 tables, 256B rows);
aggregation uses one-hot selection matrices (DVE is_equal vs an iota
row) contracted on the tensor engine with PSUM accumulation per window.
Between layers, h is published as bf16 in 4 quarter-pieces via 4
AllGathers that overlap layer-1 compute.
"""
import os
import sys

sys.path.insert(0, "/opt/trn_rl_repo")

import numpy as np
import ml_dtypes

import concourse.bacc as bacc
import concourse.tile as tile
from concourse import bass, mybir
from concourse.bass_utils import run_bass_kernel_spmd
from concourse.masks import make_identity

F32 = mybir.dt.float32
BF16 = mybir.dt.bfloat16
I16 = mybir.dt.int16
BF = ml_dtypes.bfloat16

P = 128          # partition width / chunk size / feature dim
D = 128          # feature dim
NCORES = 8
NQ = 4           # src banks (= table quarters; int16 index limit)
PAD_DOFF = 300.0  # dstoff value for pad slots (matches no iota lane)
SUBG = 512        # chunks per dma_gather instruction (512 = never split;
                  # 8 (1024-idx sub-gathers) measured slower on HW at mid scale)

LAST_EXEC_NS = None
LAST_RESULTS = None
LAST_NC = None
LAST_IN_MAPS = None


class Cfg:
    def __init__(self, n_nodes, n_edges):
        assert n_nodes % (NCORES * NQ) == 0
        self.N = n_nodes
        self.E = n_edges
        self.NSH = n_nodes // NCORES          # nodes per dst shard
        self.QR = self.NSH // NQ              # real rows per quarter
        self.WQ = -(-self.QR // P)            # windows per quarter
        self.QP = self.WQ * P                 # padded rows per quarter
        self.W = NQ * self.WQ                 # windows per core
        self.SGW = 5 if self.WQ % 5 == 0 else 1   # windows per super-group
        assert self.WQ % self.SGW == 0
        self.NSG = self.W // self.SGW
        self.BR = NCORES * self.QP            # rows per bank
        assert self.BR - 1 <= 32767, "bank exceeds int16 index range"
        self.VPAD = NQ * self.BR              # padded table rows


def _map_nodes(cfg, node):
    """Map raw node ids -> (bank, in-bank row) of the quarter-major table."""
    c = node // cfg.NSH
    local = node % cfg.NSH
    q = np.minimum(local // cfg.QR, NQ - 1)
    r = local - q * cfg.QR
    return q, c * cfg.QP + r


def _host_prep(cfg, x, edge_index):
    """Build per-core gather-index / dstoff streams and the shared layout."""
    src = np.asarray(edge_index[0], dtype=np.int64)
    dst = np.asarray(edge_index[1], dtype=np.int64)
    E = src.shape[0]

    core = dst // cfg.NSH
    dl = dst % cfg.NSH
    qd = np.minimum(dl // cfg.QR, NQ - 1)
    rd = dl - qd * cfg.QR
    win = qd * cfg.WQ + rd // P            # window within core
    doff = rd % P                          # one-hot lane within window
    bank, idx16 = _map_nodes(cfg, src)

    # counts per (core, window, bank)
    key = ((core * cfg.W + win) * NQ + bank).astype(np.int64)
    counts = np.bincount(key, minlength=NCORES * cfg.W * NQ).reshape(
        NCORES, cfg.W, NQ
    )
    kwb = -(-counts.max(axis=0) // P)      # [W, NQ] chunks, shared layout
    kwb[:, 0] = np.maximum(kwb[:, 0], 1)   # every window needs >=1 chunk

    # stream order: for sg: for b: for w in sg: for k in K_wb[w,b]
    order = []                              # (w, b) in stream order
    for s in range(cfg.NSG):
        ws = range(s * cfg.SGW, (s + 1) * cfg.SGW)
        for b in range(NQ):
            for w in ws:
                order.append((w, b))
    chunk_base = {}                         # (w,b) -> first chunk idx in stream
    nch = 0
    for (w, b) in order:
        chunk_base[(w, b)] = nch
        nch += int(kwb[w, b])
    total_slots = nch * P

    # slot position of every edge within its core's stream
    edge_sort = np.lexsort((src, key))      # group by (core, win, bank)
    ks = key[edge_sort]
    group_start = np.searchsorted(ks, np.arange(NCORES * cfg.W * NQ), side="left")
    rank_within = np.arange(E) - group_start[ks]
    cw = ks // NQ
    wb_w = (cw % cfg.W).astype(np.int64)
    wb_b = (ks % NQ).astype(np.int64)
    base_arr = np.zeros((cfg.W, NQ), dtype=np.int64)
    for (w, b), cb in chunk_base.items():
        base_arr[w, b] = cb * P
    slot = base_arr[wb_w, wb_b] + rank_within
    edge_core = (ks // (cfg.W * NQ)).astype(np.int64)

    idx_streams = np.zeros((NCORES, total_slots), dtype=np.int16)
    doff_streams = np.full((NCORES, total_slots), PAD_DOFF, dtype=np.float32)
    idx_streams[edge_core, slot] = idx16[edge_sort].astype(np.int16)
    doff_streams[edge_core, slot] = doff[edge_sort].astype(np.float32)

    # idx wrap16 layout [128, total/16]; doff [128, nch] chunk-major
    idxw = np.ascontiguousarray(
        np.tile(
            idx_streams.reshape(NCORES, total_slots // 16, 16).transpose(0, 2, 1),
            (1, 8, 1),
        )
    )
    doffc = np.ascontiguousarray(
        doff_streams.reshape(NCORES, nch, P).transpose(0, 2, 1)
    )

    # bf16 gather table for layer 1, quarter-major layout
    xpad = np.zeros((cfg.VPAD, D), dtype=BF)
    nodes = np.arange(cfg.N, dtype=np.int64)
    qn, rn = _map_nodes(cfg, nodes)
    xpad[qn * cfg.BR + rn] = x.astype(BF)

    # per-core raw x shard in padded (quarter-major) local layout
    xmy = np.zeros((NCORES, NQ * cfg.QP, D), dtype=np.float32)
    c_all = nodes // cfg.NSH
    local = nodes % cfg.NSH
    q_all = np.minimum(local // cfg.QR, NQ - 1)
    r_all = local - q_all * cfg.QR
    xmy[c_all, q_all * cfg.QP + r_all] = x

    return dict(
        kwb=kwb,
        chunk_base=chunk_base,
        order=order,
        nch=nch,
        idxw=idxw,
        doffc=doffc,
        xpad=xpad,
        xmy=xmy,
    )


def _build_program(cfg, kwb, nch):
    """Emit the SPMD Bass program. kwb: [W, NQ] chunk counts (static)."""
    nc = bacc.Bacc(None, target_bir_lowering=False, debug=False)
    kwb = np.asarray(kwb)

    xpad_t = nc.declare_dram_parameter("xpad", [cfg.VPAD, D], BF16, isOutput=False)
    xmy_t = nc.declare_dram_parameter("xmy", [NQ * cfg.QP, D], F32, isOutput=False)
    idxw_t = nc.declare_dram_parameter(
        "idxw", [P, (nch * P) // 16], I16, isOutput=False
    )
    doff_t = nc.declare_dram_parameter("doffc", [P, nch], F32, isOutput=False)
    iota_t = nc.declare_dram_parameter("iota", [P, P], BF16, isOutput=False)
    wl1_t = nc.declare_dram_parameter("W_l1", [D, D], F32, isOutput=False)
    wr1_t = nc.declare_dram_parameter("W_r1", [D, D], F32, isOutput=False)
    wl2_t = nc.declare_dram_parameter("W_l2", [D, D], F32, isOutput=False)
    wr2_t = nc.declare_dram_parameter("W_r2", [D, D], F32, isOutput=False)
    bl1_t = nc.declare_dram_parameter("b_l1", [D, 1], F32, isOutput=False)
    bl2_t = nc.declare_dram_parameter("b_l2", [D, 1], F32, isOutput=False)
    out_t = nc.declare_dram_parameter("out", [NQ * cfg.QP, D], F32, isOutput=True)

    # chunk index in the stream for (w, b, k)
    base_arr = np.zeros((cfg.W, NQ), dtype=np.int64)
    nch_chk = 0
    for s in range(cfg.NSG):
        ws = range(s * cfg.SGW, (s + 1) * cfg.SGW)
        for b in range(NQ):
            for w in ws:
                base_arr[w, b] = nch_chk
                nch_chk += int(kwb[w, b])
    assert nch_chk == nch

    # per-window (bank, k) sequence for start/stop flags
    win_seq = []
    for w in range(cfg.W):
        seq = [(b, k) for b in range(NQ) for k in range(int(kwb[w, b]))]
        win_seq.append(seq)

    assert cfg.SGW <= 5, "psum banks: need one per open window group"

    with tile.TileContext(nc, trace_sim=bool(os.environ.get("GNN_TRACE_SIM"))) as tc:
        with (
            tc.tile_pool(name="const", bufs=1) as cp,
            tc.tile_pool(name="gather", bufs=3) as gp,
            tc.tile_pool(name="onehot", bufs=4) as op_,
            tc.tile_pool(name="wstage", bufs=3) as wp,
            tc.tile_pool(name="mps", bufs=1, space="PSUM") as mpp,
            tc.tile_pool(name="wps", bufs=2, space="PSUM") as wpp,
            tc.tile_pool(name="dram", bufs=1, space="DRAM") as dp,
        ):
            ident = cp.tile([P, P], F32)
            make_identity(nc, ident[:])
            ones_bf = cp.tile([P, 1], BF16)
            nc.vector.memset(ones_bf[:], 1.0)
            iota_s = cp.tile([P, P], BF16)
            nc.sync.dma_start(iota_s[:], iota_t[:, :])
            wl1 = cp.tile([D, D], F32)
            nc.sync.dma_start(wl1[:], wl1_t[:, :])
            wr1 = cp.tile([D, D], F32)
            nc.sync.dma_start(wr1[:], wr1_t[:, :])
            wl2 = cp.tile([D, D], F32)
            nc.sync.dma_start(wl2[:], wl2_t[:, :])
            wr2 = cp.tile([D, D], F32)
            nc.sync.dma_start(wr2[:], wr2_t[:, :])
            bl1 = cp.tile([D, 1], F32)
            nc.sync.dma_start(bl1[:], bl1_t[:, :])
            bl2 = cp.tile([D, 1], F32)
            nc.sync.dma_start(bl2[:], bl2_t[:, :])
            idx_s = cp.tile([P, (nch * P) // 16], I16)
            nc.sync.dma_start(idx_s[:], idxw_t[:, :])
            doff_s = cp.tile([P, nch], F32)
            nc.sync.dma_start(doff_s[:], doff_t[:, :])
            rvec = cp.tile([P, cfg.W], F32)      # 1/max(cnt,1) per window
            tc.strict_bb_all_engine_barrier()

            h_my = dp.tile([NQ * cfg.QP, D], F32)
            hpub = [dp.tile([cfg.QP, D], BF16, name=f"hpub{q}") for q in range(NQ)]
            htbl = [
                dp.tile([cfg.BR, D], BF16, addr_space="Shared", name=f"htbl{q}")
                for q in range(NQ)
            ]

            for layer in (1, 2):
                if layer == 1:
                    tables = [
                        xpad_t[b * cfg.BR : (b + 1) * cfg.BR, :] for b in range(NQ)
                    ]
                    root, wl, wr, bl = xmy_t, wl1, wr1, bl1
                    act = mybir.ActivationFunctionType.Relu
                else:
                    tables = [htbl[b][:, :] for b in range(NQ)]
                    root, wl, wr, bl = h_my, wl2, wr2, bl2
                    act = mybir.ActivationFunctionType.Identity

                for s in range(cfg.NSG):
                    ws = list(range(s * cfg.SGW, (s + 1) * cfg.SGW))
                    # one psum bank per window: mean cols 0:P, count col P;
                    # accumulated as a SINGLE group (one zero region each)
                    wt = [
                        mpp.tile([P, P + 1], F32, tag=f"win{wi}", space="PSUM",
                                 name=f"winps{wi}")
                        for wi in range(len(ws))
                    ]

                    def mean_slot(wi):
                        return wt[wi][:, 0:P]

                    def cnt_slot(wi):
                        return wt[wi][:, P : P + 1]

                    for b in range(NQ):
                        cb0 = base_arr[ws[0], b]
                        csb = sum(int(kwb[w, b]) for w in ws)
                        if csb == 0:
                            continue
                        gb = gp.tile([P, csb * P], BF16, tag="gb")
                        gb3 = gb[:].rearrange("p (g e) -> p g e", e=P)
                        for sub in range(0, csb, SUBG):
                            csub = min(SUBG, csb - sub)
                            nc.gpsimd.dma_gather(
                                out_ap=gb3[:, sub : sub + csub, :],
                                in_ap=tables[b],
                                idxs_ap=idx_s[
                                    :, (cb0 + sub) * 8 : (cb0 + sub + csub) * 8
                                ],
                                num_idxs=csub * P,
                                num_idxs_reg=csub * P,
                                elem_size=D,
                                single_packet=False,
                            )
                        if layer == 1:
                            nc.scalar.activation(
                                gb[:], gb[:], mybir.ActivationFunctionType.Relu
                            )
                        cc = 0
                        for wi, w in enumerate(ws):
                            for k in range(int(kwb[w, b])):
                                col = base_arr[w, b] + k
                                st = op_.tile([P, P], BF16, tag="sel")
                                nc.vector.tensor_scalar(
                                    out=st[:],
                                    in0=iota_s[:],
                                    scalar1=doff_s[:, col : col + 1],
                                    scalar2=None,
                                    op0=mybir.AluOpType.is_equal,
                                )
                                first = win_seq[w][0] == (b, k)
                                last = win_seq[w][-1] == (b, k)
                                nc.tensor.matmul(
                                    out=mean_slot(wi),
                                    lhsT=st[:],
                                    rhs=gb[:, cc * P : (cc + 1) * P],
                                    start=first,
                                    stop=(last and layer == 2),
                                    skip_group_check=True,
                                )
                                if layer == 1:
                                    nc.tensor.matmul(
                                        out=cnt_slot(wi),
                                        lhsT=st[:],
                                        rhs=ones_bf[:, :1],
                                        start=False,
                                        stop=last,
                                        skip_group_check=True,
                                    )
                                cc += 1

                    # weight stage for this SG
                    for wi, w in enumerate(ws):
                        if layer == 1:
                            nc.vector.tensor_scalar_max(
                                rvec[:, w : w + 1], cnt_slot(wi), 1.0
                            )
                            nc.vector.reciprocal(
                                rvec[:, w : w + 1], rvec[:, w : w + 1]
                            )
                        mean_sb = wp.tile([P, P], F32, tag="mean_sb")
                        nc.vector.tensor_scalar_mul(
                            mean_sb[:], mean_slot(wi), rvec[:, w : w + 1]
                        )
                        tps = wpp.tile([P, 4 * P], F32, tag="tps", space="PSUM")
                        nc.tensor.transpose(
                            out=tps[:, 0:P], in_=mean_sb[:], identity=ident[:]
                        )
                        meanT = wp.tile([P, P], F32, tag="meanT")
                        nc.vector.tensor_copy(meanT[:], tps[:, 0:P])
                        root_sb = wp.tile([P, P], F32, tag="root")
                        nc.sync.dma_start(
                            root_sb[:], root[w * P : (w + 1) * P, :]
                        )
                        nc.tensor.transpose(
                            out=tps[:, P : 2 * P], in_=root_sb[:], identity=ident[:]
                        )
                        rootT = wp.tile([P, P], F32, tag="rootT")
                        nc.vector.tensor_copy(rootT[:], tps[:, P : 2 * P])
                        zps = wpp.tile([P, P], F32, tag="zps", space="PSUM",
                                       bufs=1)
                        nc.tensor.matmul(
                            out=zps[:], lhsT=wl[:], rhs=meanT[:],
                            start=True, stop=False,
                        )
                        nc.tensor.matmul(
                            out=zps[:], lhsT=wr[:], rhs=rootT[:],
                            start=False, stop=True,
                        )
                        hT = wp.tile([P, P], F32, tag="hT")
                        nc.scalar.activation(hT[:], zps[:], act, bias=bl[:, :1])
                        nc.tensor.transpose(
                            out=tps[:, 2 * P : 3 * P], in_=hT[:], identity=ident[:]
                        )
                        if layer == 1:
                            h_sb = wp.tile([P, P], F32, tag="h_sb")
                            nc.vector.tensor_copy(h_sb[:], tps[:, 2 * P : 3 * P])
                            nc.sync.dma_start(
                                h_my[w * P : (w + 1) * P, :], h_sb[:]
                            )
                            hpub_sb = wp.tile([P, P], BF16, tag="hpub_sb")
                            nc.vector.tensor_copy(
                                hpub_sb[:], tps[:, 2 * P : 3 * P]
                            )
                            q, wq = w // cfg.WQ, w % cfg.WQ
                            nc.sync.dma_start(
                                hpub[q][wq * P : (wq + 1) * P, :], hpub_sb[:]
                            )
                        else:
                            o_sb = wp.tile([P, P], F32, tag="o_sb")
                            nc.vector.tensor_copy(o_sb[:], tps[:, 2 * P : 3 * P])
                            nc.sync.dma_start(
                                out_t[w * P : (w + 1) * P, :], o_sb[:]
                            )

                    if layer == 1 and (s + 1) % (cfg.WQ // cfg.SGW) == 0:
                        q = (s + 1) // (cfg.WQ // cfg.SGW) - 1
                        nc.gpsimd.collective_compute(
                            "AllGather",
                            mybir.AluOpType.bypass,
                            replica_groups=[list(range(NCORES))],
                            ins=[hpub[q][:].opt()],
                            outs=[htbl[q][:].opt()],
                        )
    nc.finalize()
    return nc


def kernel(x, edge_index, W_l1, b_l1, W_r1, W_l2, b_l2, W_r2):
    x = np.asarray(x, dtype=np.float32)
    cfg = Cfg(x.shape[0], np.asarray(edge_index).shape[1])
    prep = _host_prep(cfg, x, edge_index)

    iota = np.tile(np.arange(P, dtype=np.float32), (P, 1)).astype(BF)
    shared = dict(
        xpad=prep["xpad"],
        iota=iota,
        W_l1=np.asarray(W_l1, np.float32),
        W_r1=np.asarray(W_r1, np.float32),
        W_l2=np.asarray(W_l2, np.float32),
        W_r2=np.asarray(W_r2, np.float32),
        b_l1=np.asarray(b_l1, np.float32).reshape(D, 1),
        b_l2=np.asarray(b_l2, np.float32).reshape(D, 1),
    )
    in_maps = []
    for c in range(NCORES):
        in_maps.append(
            dict(
                shared,
                xmy=prep["xmy"][c],
                idxw=prep["idxw"][c],
                doffc=prep["doffc"][c],
            )
        )

    nc = _build_program(cfg, prep["kwb"], prep["nch"])
    res = run_bass_kernel_spmd(nc, in_maps, list(range(NCORES)))
    global LAST_EXEC_NS, LAST_RESULTS, LAST_NC, LAST_IN_MAPS
    LAST_EXEC_NS = res.exec_time_ns
    LAST_RESULTS = res
    LAST_NC = nc
    LAST_IN_MAPS = in_maps

    out = np.empty((cfg.N, D), dtype=np.float32)
    nodes = np.arange(cfg.N, dtype=np.int64)
    c_all = nodes // cfg.NSH
    local = nodes % cfg.NSH
    q_all = np.minimum(local // cfg.QR, NQ - 1)
    r_all = local - q_all * cfg.QR
    for c in range(NCORES):
        m = c_all == c
        out[nodes[m]] = res.results[c]["out"][(q_all * cfg.QP + r_all)[m]]
    return out



# revision 8
# speedup vs baseline: 2.6219x; 2.6219x over previous
"""Trainium2 Bass kernel for the 2-layer GraphSAGE encoder (mean aggregation).

Computation (see reference):
  h   = relu(mean_agg(relu(x)[src] by dst) @ W_l1 + b_l1 + x @ W_r1)
  out =      mean_agg(h[src]       by dst) @ W_l2 + b_l2 + h @ W_r2

Distribution (src-sharded): nodes are sliced 1/8 per core; each core keeps
its slice's features resident in SBUF and handles exactly the edges whose
src falls in its slice.  Messages are fetched with SBUF-source dma_gather
(transpose mode -> feature-major chunks), transformed by W_l on the tensor
engine (linearity lets W_l and the 1/deg mean scale commute with the
segment sum), aggregated per 128-node dst window via one-hot matmuls into
PSUM, and written as bf16 partial sums for all N nodes.  A ReduceScatter
(add) gives each core the complete mean@W_l rows for its own slice; the
combine (root z@W_r + bias + activation) then runs purely per-slice, and
layer 2 reuses the resulting h slice as its gather table - no AllGather
needed anywhere.

Host prep computes global in-degrees (the mean scale is folded into the
per-window flush as an activation scale) and per-core window-major edge
streams: int16 gather indices (wrap-16 layout) plus a bf16 dst-lane stream
that drives is_equal one-hot builds on the DVE.
"""
import os
import sys

sys.path.insert(0, "/opt/trn_rl_repo")

import numpy as np
import ml_dtypes

import concourse.bacc as bacc
import concourse.tile as tile
from concourse import bass, mybir
from concourse.bass_utils import run_bass_kernel_spmd
from concourse.masks import make_identity

F32 = mybir.dt.float32
BF16 = mybir.dt.bfloat16
I16 = mybir.dt.int16
BF = ml_dtypes.bfloat16

P = 128
D = 128
NCORES = 8
PAD_LANE = 255.0   # doff value for pad slots (no iota lane matches)
G = 32             # chunks per dma_gather instruction
SUB = 4            # chunks per transform-psum batch
WB = 7             # windows per staging DMA batch

LAST_EXEC_NS = None
LAST_RESULTS = None
LAST_NC = None
LAST_IN_MAPS = None


class Cfg:
    def __init__(self, n_nodes, n_edges):
        assert n_nodes % NCORES == 0
        self.N = n_nodes
        self.E = n_edges
        self.NSH = n_nodes // NCORES            # real nodes per slice
        self.NWS = -(-self.NSH // P)            # windows per slice
        self.NSHP = self.NWS * P                # padded nodes per slice
        self.NW = NCORES * self.NWS             # global windows
        self.NPAD = NCORES * self.NSHP          # padded global nodes
        assert self.NSHP - 1 <= 32767


def _host_prep(cfg, x, edge_index):
    src = np.asarray(edge_index[0], dtype=np.int64)
    dst = np.asarray(edge_index[1], dtype=np.int64)
    E = src.shape[0]

    core = src // cfg.NSH
    idx16 = (src % cfg.NSH).astype(np.int16)
    pd = (dst // cfg.NSH) * cfg.NSHP + (dst % cfg.NSH)
    w = pd // P
    lane = (pd % P).astype(np.float32)

    key = core * cfg.NW + w
    counts = np.bincount(key, minlength=NCORES * cfg.NW).reshape(NCORES, cfg.NW)
    K = np.maximum(-(-counts.max(axis=0) // P), 1)      # [NW] chunks per window
    base = np.zeros(cfg.NW + 1, np.int64)
    np.cumsum(K, out=base[1:])
    nch = int(base[-1])
    slots = nch * P

    order = np.argsort(key, kind="stable")
    ks = key[order]
    gs = np.searchsorted(ks, np.arange(NCORES * cfg.NW), side="left")
    rank = np.arange(E) - gs[ks]
    slot = base[ks % cfg.NW] * P + rank
    ecore = ks // cfg.NW

    idx_st = np.zeros((NCORES, slots), np.int16)
    dof_st = np.full((NCORES, slots), PAD_LANE, np.float32)
    idx_st[ecore, slot] = idx16[order]
    dof_st[ecore, slot] = lane[order]

    idxw = np.ascontiguousarray(
        idx_st.reshape(NCORES, slots // 16, 16).transpose(0, 2, 1)
    )                                                   # [8, 16, slots/16]
    doffc = np.ascontiguousarray(
        dof_st.reshape(NCORES, nch, P).transpose(0, 2, 1)
    ).astype(BF)                                        # [8, 128, nch]

    indeg = np.bincount(dst, minlength=cfg.N).astype(np.float64)
    rv = (1.0 / np.maximum(indeg, 1.0)).astype(np.float32)
    rvp = np.ones(cfg.NPAD, np.float32)
    nodes = np.arange(cfg.N)
    rvp[(nodes // cfg.NSH) * cfg.NSHP + nodes % cfg.NSH] = rv
    rvq = np.ascontiguousarray(rvp.reshape(cfg.NW, P).T).astype(BF)  # [128, NW]

    xsl = np.zeros((NCORES, cfg.NSHP, D), BF)
    xs = np.asarray(x, np.float32).reshape(NCORES, cfg.NSH, D)
    xsl[:, : cfg.NSH] = xs

    return dict(K=K, nch=nch, idxw=idxw, doffc=doffc, rvq=rvq, xsl=xsl)


def _build_program(cfg, K, nch):
    nc = bacc.Bacc(None, target_bir_lowering=False, debug=False)
    K = np.asarray(K)
    slots = nch * P
    NWS, NW = cfg.NWS, cfg.NW

    xsl_t = nc.declare_dram_parameter("xmy", [cfg.NSHP, D], BF16, isOutput=False)
    idxw_t = nc.declare_dram_parameter("idxw", [16, slots // 16], I16, isOutput=False)
    doff_t = nc.declare_dram_parameter("doffc", [P, nch], BF16, isOutput=False)
    rvq_t = nc.declare_dram_parameter("rvq", [P, NW], BF16, isOutput=False)
    iota_t = nc.declare_dram_parameter("iota", [P, P], BF16, isOutput=False)
    wl1_t = nc.declare_dram_parameter("W_l1", [D, D], BF16, isOutput=False)
    wr1_t = nc.declare_dram_parameter("W_r1", [D, D], BF16, isOutput=False)
    wl2_t = nc.declare_dram_parameter("W_l2", [D, D], BF16, isOutput=False)
    wr2_t = nc.declare_dram_parameter("W_r2", [D, D], BF16, isOutput=False)
    bt1_t = nc.declare_dram_parameter("bt1", [P, P], BF16, isOutput=False)
    bt2_t = nc.declare_dram_parameter("bt2", [P, P], BF16, isOutput=False)
    out_t = nc.declare_dram_parameter("out", [cfg.NSHP, D], BF16, isOutput=True)

    # chunk -> (window, first, last)
    cofw = []
    for w in range(NW):
        for k in range(int(K[w])):
            cofw.append((w, k == 0, k == int(K[w]) - 1))
    assert len(cofw) == nch

    Id = mybir.ActivationFunctionType.Identity
    Relu = mybir.ActivationFunctionType.Relu

    with tile.TileContext(nc, trace_sim=bool(os.environ.get("GNN_TRACE_SIM"))) as tc:
        with (
            tc.tile_pool(name="const", bufs=1) as cp,
            tc.tile_pool(name="gather", bufs=3) as gp,
            tc.tile_pool(name="mk", bufs=3) as mp,
            tc.tile_pool(name="sel", bufs=4) as op_,
            tc.tile_pool(name="stage", bufs=2) as sp_,
            tc.tile_pool(name="rs", bufs=2) as rp,
            tc.tile_pool(name="tf", bufs=2, space="PSUM") as tfp,
            tc.tile_pool(name="win", bufs=2, space="PSUM") as wpp,
            tc.tile_pool(name="cps", bufs=2, space="PSUM") as cpp,
            tc.tile_pool(name="tps", bufs=2, space="PSUM") as tpp,
            tc.tile_pool(name="dram", bufs=1, space="DRAM") as dp,
        ):
            ident = cp.tile([P, P], BF16)
            make_identity(nc, ident[:])
            ones_t = cp.tile([P, P], BF16)
            nc.vector.memset(ones_t[:], 1.0)
            iota_s = cp.tile([P, P], BF16)
            nc.sync.dma_start(iota_s[:], iota_t[:, :])
            wl1 = cp.tile([D, D], BF16)
            nc.sync.dma_start(wl1[:], wl1_t[:, :])
            wr1 = cp.tile([D, D], BF16)
            nc.sync.dma_start(wr1[:], wr1_t[:, :])
            wl2 = cp.tile([D, D], BF16)
            nc.sync.dma_start(wl2[:], wl2_t[:, :])
            wr2 = cp.tile([D, D], BF16)
            nc.sync.dma_start(wr2[:], wr2_t[:, :])
            bt1 = cp.tile([P, P], BF16)
            nc.sync.dma_start(bt1[:], bt1_t[:, :])
            bt2 = cp.tile([P, P], BF16)
            nc.sync.dma_start(bt2[:], bt2_t[:, :])
            rvq_b = cp.tile([P, NW], BF16)
            nc.sync.dma_start(rvq_b[:], rvq_t[:, :])
            rvq_s = cp.tile([P, NW], F32)
            nc.vector.tensor_copy(rvq_s[:], rvq_b[:])
            doff_b = cp.tile([P, nch], BF16)
            nc.sync.dma_start(doff_b[:], doff_t[:, :])
            doff_s = cp.tile([P, nch], F32)
            nc.vector.tensor_copy(doff_s[:], doff_b[:])
            idx_s = cp.tile([P, slots // 16], I16)
            for r in range(8):
                nc.sync.dma_start(idx_s[16 * r : 16 * (r + 1), :], idxw_t[:, :])
            tbl1 = cp.tile([P, NWS * D], BF16)
            nc.sync.dma_start(
                tbl1[:].rearrange("t (r e) -> t r e", e=D),
                xsl_t[:, :].rearrange("(r t) e -> t r e", t=P),
            )
            tc.strict_bb_all_engine_barrier()

            # feature-major views of the slice (roots), gather table for L2
            xT = cp.tile([P, NWS * P], BF16)
            hT = cp.tile([P, NWS * P], BF16)
            tbl2 = cp.tile([P, NWS * D], BF16)
            for w in range(NWS):
                tp = tpp.tile([P, P], BF16, tag="tps", space="PSUM")
                nc.tensor.transpose(
                    out=tp[:], in_=tbl1[:, w * P : (w + 1) * P], identity=ident[:]
                )
                nc.scalar.activation(xT[:, w * P : (w + 1) * P], tp[:], Id)
            # messages are relu(x): relu the gather table in place (roots
            # already captured in xT)
            nc.scalar.activation(tbl1[:], tbl1[:], Relu)

            PART = dp.tile([cfg.NPAD, D], BF16, name="part")
            RSOUT = dp.tile([cfg.NSHP, D], BF16, name="rsout")

            for layer in (1, 2):
                tbl = tbl1 if layer == 1 else tbl2
                zT = xT if layer == 1 else hT
                wl = wl1 if layer == 1 else wl2
                wr = wr1 if layer == 1 else wr2
                bt = bt1 if layer == 1 else bt2

                # ---- phase A: gather + transform + window aggregation ----
                cur_win = None
                stage = None
                pend = None  # (tfps, mk emitted later, chunk ids) 1-deep pipe

                def drain(pend_batch):
                    nonlocal cur_win, stage
                    mk, cids = pend_batch
                    for i, c in enumerate(cids):
                        w, first, last = cofw[c]
                        if first:
                            cur_win = wpp.tile([P, P], F32, tag="win",
                                               space="PSUM")
                        sel = op_.tile([P, P], BF16, tag="sel")
                        nc.vector.tensor_scalar(
                            out=sel[:],
                            in0=iota_s[:],
                            scalar1=doff_s[:, c : c + 1],
                            scalar2=None,
                            op0=mybir.AluOpType.is_equal,
                        )
                        nc.tensor.matmul(
                            out=cur_win[:],
                            lhsT=sel[:],
                            rhs=mk[:, i * P : (i + 1) * P],
                            start=first,
                            stop=last,
                            skip_group_check=True,
                        )
                        if last:
                            if w % WB == 0:
                                stage = sp_.tile([P, WB * P], BF16, tag="st")
                            nc.scalar.activation(
                                stage[:, (w % WB) * P : (w % WB + 1) * P],
                                cur_win[:],
                                Id,
                                scale=rvq_s[:, w : w + 1],
                            )
                            if w % WB == WB - 1 or w == NW - 1:
                                w0 = (w // WB) * WB
                                nc.sync.dma_start(
                                    PART[w0 * P : (w + 1) * P, :]
                                    .rearrange("(w l) g -> l w g", l=P),
                                    stage[:, : (w - w0 + 1) * P],
                                )

                for c0 in range(0, nch, G):
                    gcnt = min(G, nch - c0)
                    gb = gp.tile([P, gcnt * P], BF16, tag="gb")
                    nc.gpsimd.dma_gather(
                        out_ap=gb[:].rearrange("p (o e) -> p o e", o=1),
                        in_ap=tbl[:],
                        idxs_ap=idx_s[:, c0 * 8 : (c0 + gcnt) * 8],
                        num_idxs=gcnt * P,
                        num_idxs_reg=gcnt * P,
                        elem_size=D,
                        transpose=True,
                        sbuf_tokens_per_rank=P,
                        sbuf_free_dim_per_rank=2 * D,
                        sbuf_free_dim_pad_per_rank=0,
                        sbuf_byte_offset=0,
                        single_packet=False,
                    )
                    for s0 in range(0, gcnt, SUB):
                        scnt = min(SUB, gcnt - s0)
                        tfps = tfp.tile([P, SUB * P], F32, tag="tf", space="PSUM")
                        for i in range(scnt):
                            nc.tensor.matmul(
                                out=tfps[:, i * P : (i + 1) * P],
                                lhsT=gb[:, (s0 + i) * P : (s0 + i + 1) * P],
                                rhs=wl[:],
                                start=True,
                                stop=True,
                                skip_group_check=True,
                            )
                        mk = mp.tile([P, SUB * P], BF16, tag="mk")
                        nc.scalar.activation(
                            mk[:, : scnt * P], tfps[:, : scnt * P], Id
                        )
                        if pend is not None:
                            drain(pend)
                        pend = (mk, list(range(c0 + s0, c0 + s0 + scnt)))
                if pend is not None:
                    drain(pend)
                    pend = None

                # ---- ReduceScatter partial sums across cores ----
                nc.gpsimd.collective_compute(
                    "ReduceScatter",
                    mybir.AluOpType.add,
                    replica_groups=[list(range(NCORES))],
                    ins=[PART[:, :].opt()],
                    outs=[RSOUT[:, :].opt()],
                )

                # ---- phase C: combine own slice ----
                for q0 in range(0, NWS, WB):
                    qc = min(WB, NWS - q0)
                    rs = rp.tile([P, WB * P], BF16, tag="rs")
                    nc.sync.dma_start(
                        rs[:, : qc * P].rearrange("l (w g) -> l w g", g=D),
                        RSOUT[q0 * P : (q0 + qc) * P, :]
                        .rearrange("(w l) g -> l w g", l=P),
                    )
                    if layer == 2:
                        ost = sp_.tile([P, WB * P], BF16, tag="ost", name="ost")
                    else:
                        ost = None
                    for i in range(qc):
                        w = q0 + i
                        cps = cpp.tile([P, P], F32, tag="cps", space="PSUM")
                        nc.tensor.matmul(
                            out=cps[:], lhsT=ident[:],
                            rhs=rs[:, i * P : (i + 1) * P],
                            start=True, stop=False, skip_group_check=True,
                        )
                        nc.tensor.matmul(
                            out=cps[:], lhsT=ones_t[:], rhs=bt[:],
                            start=False, stop=False, skip_group_check=True,
                        )
                        nc.tensor.matmul(
                            out=cps[:], lhsT=zT[:, w * P : (w + 1) * P],
                            rhs=wr[:],
                            start=False, stop=True, skip_group_check=True,
                        )
                        if layer == 1:
                            nc.scalar.activation(
                                tbl2[:, w * P : (w + 1) * P], cps[:], Relu
                            )
                            tp = tpp.tile([P, P], BF16, tag="tps", space="PSUM")
                            nc.tensor.transpose(
                                out=tp[:], in_=tbl2[:, w * P : (w + 1) * P],
                                identity=ident[:],
                            )
                            nc.scalar.activation(
                                hT[:, w * P : (w + 1) * P], tp[:], Id
                            )
                        else:
                            nc.scalar.activation(
                                ost[:, i * P : (i + 1) * P], cps[:], Id
                            )
                    if layer == 2:
                        nc.sync.dma_start(
                            out_t[q0 * P : (q0 + qc) * P, :]
                            .rearrange("(w l) g -> l w g", l=P),
                            ost[:, : qc * P].rearrange("l (w g) -> l w g", g=D),
                        )
    nc.finalize()
    return nc


def kernel(x, edge_index, W_l1, b_l1, W_r1, W_l2, b_l2, W_r2):
    x = np.asarray(x, dtype=np.float32)
    cfg = Cfg(x.shape[0], np.asarray(edge_index).shape[1])
    prep = _host_prep(cfg, x, edge_index)

    iota = np.tile(np.arange(P, dtype=np.float32), (P, 1)).astype(BF)
    bt1 = np.tile(np.asarray(b_l1, np.float32) / P, (P, 1)).astype(BF)
    bt2 = np.tile(np.asarray(b_l2, np.float32) / P, (P, 1)).astype(BF)
    shared = dict(
        iota=iota,
        W_l1=np.asarray(W_l1, np.float32).astype(BF),
        W_r1=np.asarray(W_r1, np.float32).astype(BF),
        W_l2=np.asarray(W_l2, np.float32).astype(BF),
        W_r2=np.asarray(W_r2, np.float32).astype(BF),
        bt1=bt1,
        bt2=bt2,
    )
    in_maps = []
    for c in range(NCORES):
        in_maps.append(
            dict(
                shared,
                xmy=prep["xsl"][c],
                idxw=prep["idxw"][c],
                doffc=prep["doffc"][c],
                rvq=prep["rvq"],
            )
        )

    nc = _build_program(cfg, prep["K"], prep["nch"])
    res = run_bass_kernel_spmd(nc, in_maps, list(range(NCORES)))
    global LAST_EXEC_NS, LAST_RESULTS, LAST_NC, LAST_IN_MAPS
    LAST_EXEC_NS = res.exec_time_ns
    LAST_RESULTS = res
    LAST_NC = nc
    LAST_IN_MAPS = in_maps

    out = np.empty((cfg.N, D), dtype=np.float32)
    for c in range(NCORES):
        out[c * cfg.NSH : (c + 1) * cfg.NSH] = (
            res.results[c]["out"][: cfg.NSH].astype(np.float32)
        )
    return out
